# revision 16
# baseline (speedup 1.0000x reference)
"""Trainium2 Bass kernel: BiDAF-style context-query attention (nn_CQattn).

Reference (per batch b):
    S    = (C@w1)[:,None] + (Q@w2)[None,:] + (C*w3) @ Q.T        # [N, M]
    S1   = softmax_m(S + NEG*Qmask[None,:])                      # row softmax
    S2   = softmax_n(S + NEG*Cmask[:,None])                      # col softmax
    A    = S1 @ Q                                                # [N, D]
    Bout = S1 @ (S2.T @ C)                                       # [N, D]

Device algebra (per batch; E0 = exp(dot3), dot3 = (C*w3) @ Q.T):
    f2[n] = exp(c1[n]),  f1[m] = exp(q2[m] + NEG*Qmask[m]),  z[n] = 1-Cmask[n]
    E2    = exp(dot3 + c1[n]) = E0 * f2[n]           (ACT bias, per-partition)
    E2T   = transpose(E2)                             (PE transpose, 1 c/row)
    c2'   = E2.T @ z  ;  T' = E2.T @ (z*C)            (col softmax numerators;
                                                       Cmask applied via the
                                                       zeroed rhs, NOT via f2,
                                                       so E2 rows stay nonzero
                                                       for the S1 path)
    Tf    = T' * (f1[m]/c2'[m])                       (= diag(f1) @ T)
    r1f2  = E2T @ f1  (= f2[n] * rowsum1)             (f2 cancels in the ratio)
    A     = (E2T.T @ Qf)  / r1f2,   Qf = f1[m]*Q      (per-partition scale)
    Bout  = (E2T.T @ Tf)  / r1f2
The single exp + PE transpose replaces the baseline's second dot3 matmul
pass (-64 big matmuls/batch); C^T, Q*w3^T, Qf, c1m, f1 are precomputed on
the host and shipped as inputs (transposes/scalings are O(N*D) host work).

All matmul operands are bf16 (1 cycle/row on the PE, half the DMA/SBUF
traffic of fp32r); accumulation stays fp32 in PSUM, biases/scales fp32.
Outputs are written bf16 and upcast on the host (measured rel_fro ~2e-3,
gate is 2e-2).

Sharding: data-parallel over batch: 32 batches / 8 cores = 4 per core.
Self-contained: shapes hardcoded; no sibling imports.

Toolchain note: the walrus build in this container accepts at most one
sem-wait per instruction, while Tile's scheduler attaches several; the
_patch_tile_drain_wait_split hook below splits excess waits onto
same-engine NOPs (required for ANY Tile kernel to compile here).
"""

import numpy as np

B, N, M, D = 32, 2048, 512, 512
NCORES = 8
BPC = B // NCORES  # batches per core
NEG = -1e30

NT = N // 128  # 16 n-tiles
MT = M // 128  # 4 m-tiles
DT = D // 128  # 4 d-tiles
NQ = N // 512  # 4 groups of 4 n-tiles


def _patch_tile_drain_wait_split():
    """The stock Tile kernel-tail drain carries one sem-wait per still-pending
    proc on a single InstDrain; the walrus build in this container rejects >1
    sync wait per instruction ("Too many sync wait commands").  Split the
    excess waits onto dedicated sync-engine NOPs emitted right after the
    drain (they still precede the all-engine barrier, preserving the
    everything-done-before-teardown guarantee)."""
    import concourse.mybir as mybir
    import concourse.tile as tile

    if getattr(tile.TileContext, "_drain_wait_split_patched", False):
        return

    orig_add = tile.TileContext._add_instruction

    def _add_instruction(self, inst):
        si = inst.sync_info
        waits = list(si.on_wait) if si and si.on_wait else []
        if len(waits) > 1 and inst.engine != mybir.EngineType.Unassigned:
            for w in waits[:-1]:
                nop = mybir.InstNoOp(
                    name=self.nc.get_next_instruction_name(), ins=[], outs=[]
                )
                nop.engine = inst.engine
                nop.sync_info = mybir.SyncInfo(on_wait=[w], on_update=[])
                orig_add(self, nop)
            inst.sync_info = mybir.SyncInfo(
                on_wait=[waits[-1]],
                on_update=list(si.on_update) if si.on_update else [],
            )
        orig_add(self, inst)

    tile.TileContext._add_instruction = _add_instruction

    def _drain_and_barrier(self, tick_clock, wait_clock):
        nc = self.nc
        drain_inst = nc.sync.drain()
        wait_clock.add_sem_waits(
            drain_inst.ins, tile.ScopedClock({None: tick_clock.global_clock})
        )
        si = drain_inst.ins.sync_info
        waits = list(si.on_wait) if si and si.on_wait else []
        if len(waits) > 1:
            drain_inst.ins.sync_info = mybir.SyncInfo(
                on_wait=[waits[0]],
                on_update=list(si.on_update) if si and si.on_update else [],
            )
            for w in waits[1:]:
                nop = nc.sync.nop(nofuse=True, hint="drain_wait_split")
                nop.ins.sync_info = mybir.SyncInfo(on_wait=[w], on_update=[])

        nc.all_engine_barrier()
        assert self.sems is not None
        popped = nc._tile_sem_poison_stack.pop()
        assert popped is self._sem_poison
        nc.clear_and_free_semaphores(list(self.sems.allocated().values()))
        nc.all_engine_barrier()

    tile.TileContext._drain_and_barrier = _drain_and_barrier
    tile.TileContext._drain_wait_split_patched = True


def build_nc(n_reps=1):
    import concourse.bass as bass
    import concourse.mybir as mybir
    import concourse.tile as tile

    _patch_tile_drain_wait_split()

    f32 = mybir.dt.float32
    bf16 = mybir.dt.bfloat16
    AF = mybir.ActivationFunctionType

    nc = bass.Bass()
    CT_d = nc.dram_tensor("CT", [BPC, D, N], bf16, kind="ExternalInput")
    Cn_d = nc.dram_tensor("Cn", [BPC, N, D], bf16, kind="ExternalInput")
    Qf_d = nc.dram_tensor("Qf", [BPC, M, D], bf16, kind="ExternalInput")
    QwT_d = nc.dram_tensor("QwT", [BPC, D, M], bf16, kind="ExternalInput")
    c1m_d = nc.dram_tensor("c1m", [128, BPC, NT], f32, kind="ExternalInput")
    f1f_d = nc.dram_tensor("f1f", [128, BPC, MT], f32, kind="ExternalInput")
    f1b_d = nc.dram_tensor("f1b", [128, BPC, MT], bf16, kind="ExternalInput")
    id_d = nc.dram_tensor("ident", [128, 128], bf16, kind="ExternalInput")
    zb_d = nc.dram_tensor("zb", [128, BPC, NT], bf16, kind="ExternalInput")
    A_d = nc.dram_tensor("A", [BPC, N, D], bf16, kind="ExternalOutput")
    Bo_d = nc.dram_tensor("Bout", [BPC, N, D], bf16, kind="ExternalOutput")

    mm = nc.tensor.matmul
    # E2 -> E2T via the DMA transpose crossbar instead of PE transposes:
    # measured WORSE (64 extra DMA instrs/batch at ~625ns HWDGE overhead
    # each, and the PE gaps drop it out of max p-state). Keep False.
    TR_VIA_DMA = False

    with tile.TileContext(nc) as tc:
        with (
            tc.tile_pool(name="const", bufs=1) as constp,
            tc.tile_pool(name="ctp", bufs=2) as ctpool,
            tc.tile_pool(name="cnp", bufs=2) as cnpool,
            tc.tile_pool(name="qfp", bufs=2) as qfpool,
            tc.tile_pool(name="qwp", bufs=2) as qwpool,
            tc.tile_pool(name="e2p", bufs=20) as e2pool,
            tc.tile_pool(name="e2tp", bufs=6) as e2tpool,
            tc.tile_pool(name="tfp", bufs=6) as tfpool,
            tc.tile_pool(name="smallp", bufs=24) as smallpool,
            tc.tile_pool(name="stagep", bufs=4) as stagepool,
            tc.tile_pool(name="psbig", bufs=6, space="PSUM") as psb,
            tc.tile_pool(name="pssmall", bufs=2, space="PSUM") as pss,
        ):
            ident = constp.tile([128, 128], bf16, name="ident")
            nc.sync.dma_start(ident[:], id_d[:])
            zb = constp.tile([128, BPC, NT], bf16, name="zb")
            nc.sync.dma_start(zb[:], zb_d[:])
            c1m = constp.tile([128, BPC, NT], f32, name="c1m")
            nc.sync.dma_start(c1m[:], c1m_d[:])
            f1f = constp.tile([128, BPC, MT], f32, name="f1f")
            nc.sync.dma_start(f1f[:], f1f_d[:])
            f1b = constp.tile([128, BPC, MT], bf16, name="f1b")
            nc.sync.dma_start(f1b[:], f1b_d[:])

            for b in [b for _ in range(n_reps) for b in range(BPC)]:
                ct = ctpool.tile([128, DT, N], bf16, name="CT", tag="CT")
                nc.sync.dma_start(
                    ct[:], CT_d[b].rearrange("(j p) n -> p j n", p=128)
                )
                cn = cnpool.tile([128, NT, D], bf16, name="Cn", tag="Cn")
                nc.sync.dma_start(
                    cn[:], Cn_d[b].rearrange("(s p) d -> p s d", p=128)
                )
                qf = qfpool.tile([128, MT, D], bf16, name="Qf", tag="Qf")
                nc.sync.dma_start(
                    qf[:], Qf_d[b].rearrange("(s p) d -> p s d", p=128)
                )
                qwt = qwpool.tile([128, DT, M], bf16, name="QwT", tag="QwT")
                nc.sync.dma_start(
                    qwt[:], QwT_d[b].rearrange("(j p) m -> p j m", p=128)
                )

                # ---- E2[t] = exp(dot3 + c1m[n]) [16 x [128n, 512m] bf16],
                # with E2T transposes of group tq-1 interleaved behind the
                # dot3 matmuls of group tq to keep the PE dependency-free.
                e2_tiles = [
                    e2pool.tile([128, M], bf16, name=f"E2_{t}", tag="E2")
                    for t in range(NT)
                ]
                e2t_tiles = [
                    e2tpool.tile([128, N], bf16, name=f"E2T_{u}", tag="E2T")
                    for u in range(MT)
                ]

                def tr_group(tq):
                    # transpose the 4 n-tiles of group tq into all 4 E2T tiles
                    if TR_VIA_DMA:
                        for s in range(4):
                            t = tq * 4 + s
                            for u in range(MT):
                                nc.sync.dma_start_transpose(
                                    out=e2t_tiles[u][:, t * 128 : (t + 1) * 128],
                                    in_=e2_tiles[t][:, u * 128 : (u + 1) * 128],
                                )
                        return
                    for u in range(MT):
                        pst = psb.tile([128, 512], bf16, name="ps_tr", tag="psb")
                        for s in range(4):
                            t = tq * 4 + s
                            nc.tensor.transpose(
                                pst[:, s * 128 : (s + 1) * 128],
                                e2_tiles[t][:, u * 128 : (u + 1) * 128],
                                ident[:],
                            )
                        nc.vector.tensor_copy(
                            e2t_tiles[u][:, tq * 512 : (tq + 1) * 512], pst[:]
                        )

                for tq in range(NQ):
                    for s in range(4):
                        t = tq * 4 + s
                        ps = psb.tile([128, M], f32, name="ps_e2", tag="psb")
                        for j in range(DT):
                            mm(
                                ps[:],
                                ct[:, j, t * 128 : (t + 1) * 128],
                                qwt[:, j, :],
                                start=(j == 0),
                                stop=(j == DT - 1),
                            )
                        nc.scalar.activation(
                            e2_tiles[t][:], ps[:], AF.Exp, bias=c1m[:, b, t : t + 1]
                        )
                    if tq:
                        tr_group(tq - 1)

                # ---- T stage: c2' = E2.T @ 1, T' = E2.T @ C;
                # Tf = T' * f1/c2' (the f2 in E2 cancels against r1f2 below)
                tf_tiles = []
                for u in range(MT):
                    if u == 0:
                        tr_group(NQ - 1)  # last transpose group, after its exps
                    pst = psb.tile([128, D], f32, name="ps_T", tag="psb")
                    psc = pss.tile([128, 1], f32, name="ps_c2", tag="pss")
                    for t in range(NT):
                        lhsT = e2_tiles[t][:, u * 128 : (u + 1) * 128]
                        mm(
                            pst[:], lhsT, cn[:, t, :],
                            start=(t == 0), stop=(t == NT - 1),
                        )
                        mm(
                            psc[:], lhsT, zb[:, b, t : t + 1],
                            start=(t == 0), stop=(t == NT - 1),
                        )
                    rc = smallpool.tile([128, 1], f32, name="rc2", tag="small")
                    nc.vector.reciprocal(rc[:], psc[:])
                    sc = smallpool.tile([128, 1], f32, name="scT", tag="small")
                    nc.vector.tensor_scalar_mul(sc[:], f1f[:, b, u : u + 1], rc[:])
                    tfu = tfpool.tile([128, D], bf16, name="Tf", tag="Tf")
                    nc.scalar.activation(tfu[:], pst[:], AF.Copy, scale=sc[:])
                    tf_tiles.append(tfu)

                # ---- A/B: A = (E2T.T @ Qf)/r1f2, B = (E2T.T @ Tf)/r1f2
                for g in range(NT // 2):
                    ast = stagepool.tile([128, 2, D], bf16, name="Ast", tag="Ast")
                    bst = stagepool.tile([128, 2, D], bf16, name="Bst", tag="Bst")
                    for s2 in range(2):
                        t = g * 2 + s2
                        psa = psb.tile([128, D], f32, name="ps_A", tag="psb")
                        psbb = psb.tile([128, D], f32, name="ps_B", tag="psb")
                        psr = pss.tile([128, 1], f32, name="ps_r1", tag="pss")
                        for u in range(MT):
                            lhsT = e2t_tiles[u][:, t * 128 : (t + 1) * 128]
                            mm(
                                psa[:], lhsT, qf[:, u, :],
                                start=(u == 0), stop=(u == MT - 1),
                            )
                            mm(
                                psbb[:], lhsT, tf_tiles[u][:],
                                start=(u == 0), stop=(u == MT - 1),
                            )
                            mm(
                                psr[:], lhsT, f1b[:, b, u : u + 1],
                                start=(u == 0), stop=(u == MT - 1),
                            )
                        r1 = smallpool.tile([128, 1], f32, name="r1", tag="small")
                        nc.vector.reciprocal(r1[:], psr[:])
                        nc.scalar.activation(
                            ast[:, s2, :], psa[:], AF.Copy, scale=r1[:]
                        )
                        nc.vector.tensor_scalar_mul(bst[:, s2, :], psbb[:], r1[:])
                    nc.sync.dma_start(
                        A_d[b, g * 256 : (g + 1) * 256, :].rearrange(
                            "(s p) d -> p s d", p=128
                        ),
                        ast[:],
                    )
                    nc.sync.dma_start(
                        Bo_d[b, g * 256 : (g + 1) * 256, :].rearrange(
                            "(s p) d -> p s d", p=128
                        ),
                        bst[:],
                    )

    return nc


_NC = None


def _get_nc():
    global _NC
    if _NC is None:
        _NC = build_nc()
        _NC.finalize()
    return _NC


def _make_in_maps(C, Q, Cmask, Qmask, w):
    import ml_dtypes

    bf = ml_dtypes.bfloat16
    C = np.asarray(C, dtype=np.float32)
    Q = np.asarray(Q, dtype=np.float32)
    w = np.asarray(w, dtype=np.float32)
    w1, w2, w3 = w[:D], w[D : 2 * D], w[2 * D :]

    c1 = C @ w1  # [B, N]
    q2 = Q @ w2  # [B, M]
    c1m_full = c1  # S1 path is NOT masked by Cmask; Cmask enters via z below
    z_full = 1.0 - np.asarray(Cmask, dtype=np.float32)  # [B, N]; 0 = masked
    f1_full = np.exp(q2 + np.float32(NEG) * np.asarray(Qmask, dtype=np.float32))

    Cb = C.astype(bf)
    CTb = np.ascontiguousarray(Cb.transpose(0, 2, 1))
    Czb = (z_full[:, :, None] * C).astype(bf)  # masked rows zeroed, for T path
    Qfb = (f1_full[:, :, None] * Q).astype(bf)
    QwTb = np.ascontiguousarray((Q * w3[None, None, :]).astype(bf).transpose(0, 2, 1))
    ident = np.eye(128, dtype=bf)

    in_maps = []
    for c in range(NCORES):
        bs = slice(c * BPC, (c + 1) * BPC)
        c1m = np.ascontiguousarray(
            c1m_full[bs].reshape(BPC, NT, 128).transpose(2, 0, 1)
        )
        zl = z_full[bs].reshape(BPC, NT, 128).transpose(2, 0, 1)
        f1l = f1_full[bs].reshape(BPC, MT, 128).transpose(2, 0, 1)
        in_maps.append(
            {
                "CT": CTb[bs],
                "Cn": np.ascontiguousarray(Czb[bs]),
                "Qf": np.ascontiguousarray(Qfb[bs]),
                "QwT": QwTb[bs],
                "c1m": c1m,
                "f1f": np.ascontiguousarray(f1l.astype(np.float32)),
                "f1b": np.ascontiguousarray(f1l.astype(bf)),
                "ident": ident,
                "zb": np.ascontiguousarray(zl.astype(bf)),
            }
        )
    return in_maps


def run_spmd(C, Q, Cmask, Qmask, w, trace=False):
    """Returns ((A, Bout), BassKernelResults)."""
    from concourse.bass_utils import run_bass_kernel_spmd

    nc = _get_nc()
    in_maps = _make_in_maps(C, Q, Cmask, Qmask, w)
    res = run_bass_kernel_spmd(nc, in_maps, list(range(NCORES)), trace=trace)
    A = np.concatenate(
        [np.asarray(r["A"]).astype(np.float32) for r in res.results], axis=0
    )
    Bout = np.concatenate(
        [np.asarray(r["Bout"]).astype(np.float32) for r in res.results], axis=0
    )
    return (A, Bout), res


def kernel(C, Q, Cmask, Qmask, w):
    # NTFF tracing is unavailable under this container's axon relay; always
    # run the plain execute path.
    (A, Bout), _ = run_spmd(C, Q, Cmask, Qmask, w, trace=False)
    return (A, Bout)


# revision 18
# speedup vs baseline: 1.0712x; 1.0712x over previous
"""Trainium2 Bass kernel: BiDAF-style context-query attention (nn_CQattn).

Reference (per batch b):
    S    = (C@w1)[:,None] + (Q@w2)[None,:] + (C*w3) @ Q.T        # [N, M]
    S1   = softmax_m(S + NEG*Qmask[None,:])                      # row softmax
    S2   = softmax_n(S + NEG*Cmask[:,None])                      # col softmax
    A    = S1 @ Q                                                # [N, D]
    Bout = S1 @ (S2.T @ C)                                       # [N, D]

Device algebra (per batch; E0 = exp(dot3), dot3 = (C*w3) @ Q.T):
    f2[n] = exp(c1[n]),  f1[m] = exp(q2[m] + NEG*Qmask[m]),  z[n] = 1-Cmask[n]
    E2    = exp(dot3 + c1[n]) = E0 * f2[n]           (ACT bias, per-partition)
    E2T   = transpose(E2)                             (PE transpose, 1 c/row)
    c2'   = E2.T @ z  ;  T' = E2.T @ (z*C)            (col softmax numerators;
                                                       Cmask applied via the
                                                       zeroed rhs, NOT via f2,
                                                       so E2 rows stay nonzero
                                                       for the S1 path)
    Tf    = T' * (f1[m]/c2'[m])                       (= diag(f1) @ T)
    r1f2  = E2T @ f1  (= f2[n] * rowsum1)             (f2 cancels in the ratio)
    A     = (E2T.T @ Qf)  / r1f2,   Qf = f1[m]*Q      (per-partition scale)
    Bout  = (E2T.T @ Tf)  / r1f2
The single exp + PE transpose replaces the baseline's second dot3 matmul
pass (-64 big matmuls/batch); C^T, Q*w3^T, Qf, c1m, f1 are precomputed on
the host and shipped as inputs (transposes/scalings are O(N*D) host work).

All matmul operands are bf16 (1 cycle/row on the PE, half the DMA/SBUF
traffic of fp32r); accumulation stays fp32 in PSUM, biases/scales fp32.
Outputs are written bf16 and upcast on the host (measured rel_fro ~2e-3,
gate is 2e-2).

Sharding: data-parallel over batch: 32 batches / 8 cores = 4 per core.
Self-contained: shapes hardcoded; no sibling imports.

Toolchain note: the walrus build in this container accepts at most one
sem-wait per instruction, while Tile's scheduler attaches several; the
_patch_tile_drain_wait_split hook below splits excess waits onto
same-engine NOPs (required for ANY Tile kernel to compile here).
"""

import numpy as np

B, N, M, D = 32, 2048, 512, 512
NCORES = 8
BPC = B // NCORES  # batches per core
NEG = -1e30

NT = N // 128  # 16 n-tiles
MT = M // 128  # 4 m-tiles
DT = D // 128  # 4 d-tiles
NQ = N // 512  # 4 groups of 4 n-tiles


def _patch_tile_drain_wait_split():
    """The stock Tile kernel-tail drain carries one sem-wait per still-pending
    proc on a single InstDrain; the walrus build in this container rejects >1
    sync wait per instruction ("Too many sync wait commands").  Split the
    excess waits onto dedicated sync-engine NOPs emitted right after the
    drain (they still precede the all-engine barrier, preserving the
    everything-done-before-teardown guarantee)."""
    import concourse.mybir as mybir
    import concourse.tile as tile

    if getattr(tile.TileContext, "_drain_wait_split_patched", False):
        return

    orig_add = tile.TileContext._add_instruction

    def _add_instruction(self, inst):
        si = inst.sync_info
        waits = list(si.on_wait) if si and si.on_wait else []
        if len(waits) > 1 and inst.engine != mybir.EngineType.Unassigned:
            for w in waits[:-1]:
                nop = mybir.InstNoOp(
                    name=self.nc.get_next_instruction_name(), ins=[], outs=[]
                )
                nop.engine = inst.engine
                nop.sync_info = mybir.SyncInfo(on_wait=[w], on_update=[])
                orig_add(self, nop)
            inst.sync_info = mybir.SyncInfo(
                on_wait=[waits[-1]],
                on_update=list(si.on_update) if si.on_update else [],
            )
        orig_add(self, inst)

    tile.TileContext._add_instruction = _add_instruction

    def _drain_and_barrier(self, tick_clock, wait_clock):
        nc = self.nc
        drain_inst = nc.sync.drain()
        wait_clock.add_sem_waits(
            drain_inst.ins, tile.ScopedClock({None: tick_clock.global_clock})
        )
        si = drain_inst.ins.sync_info
        waits = list(si.on_wait) if si and si.on_wait else []
        if len(waits) > 1:
            drain_inst.ins.sync_info = mybir.SyncInfo(
                on_wait=[waits[0]],
                on_update=list(si.on_update) if si and si.on_update else [],
            )
            for w in waits[1:]:
                nop = nc.sync.nop(nofuse=True, hint="drain_wait_split")
                nop.ins.sync_info = mybir.SyncInfo(on_wait=[w], on_update=[])

        nc.all_engine_barrier()
        assert self.sems is not None
        popped = nc._tile_sem_poison_stack.pop()
        assert popped is self._sem_poison
        nc.clear_and_free_semaphores(list(self.sems.allocated().values()))
        nc.all_engine_barrier()

    tile.TileContext._drain_and_barrier = _drain_and_barrier
    tile.TileContext._drain_wait_split_patched = True


def build_nc(n_reps=1):
    import concourse.bass as bass
    import concourse.mybir as mybir
    import concourse.tile as tile

    _patch_tile_drain_wait_split()

    f32 = mybir.dt.float32
    bf16 = mybir.dt.bfloat16
    AF = mybir.ActivationFunctionType

    nc = bass.Bass()
    CT_d = nc.dram_tensor("CT", [BPC, D, N], bf16, kind="ExternalInput")
    Cn_d = nc.dram_tensor("Cn", [BPC, N, D], bf16, kind="ExternalInput")
    Qf_d = nc.dram_tensor("Qf", [BPC, M, D], bf16, kind="ExternalInput")
    QwT_d = nc.dram_tensor("QwT", [BPC, D, M], bf16, kind="ExternalInput")
    c1m_d = nc.dram_tensor("c1m", [128, BPC, NT], f32, kind="ExternalInput")
    f1f_d = nc.dram_tensor("f1f", [128, BPC, MT], f32, kind="ExternalInput")
    f1b_d = nc.dram_tensor("f1b", [128, BPC, MT], bf16, kind="ExternalInput")
    id_d = nc.dram_tensor("ident", [128, 128], bf16, kind="ExternalInput")
    zb_d = nc.dram_tensor("zb", [128, BPC, NT], bf16, kind="ExternalInput")
    A_d = nc.dram_tensor("A", [BPC, N, D], bf16, kind="ExternalOutput")
    Bo_d = nc.dram_tensor("Bout", [BPC, N, D], bf16, kind="ExternalOutput")

    mm = nc.tensor.matmul
    # E2 -> E2T via the DMA transpose crossbar instead of PE transposes:
    # measured WORSE (64 extra DMA instrs/batch at ~625ns HWDGE overhead
    # each, and the PE gaps drop it out of max p-state). Keep False.
    TR_VIA_DMA = False

    with tile.TileContext(nc) as tc:
        with (
            tc.tile_pool(name="const", bufs=1) as constp,
            tc.tile_pool(name="ctp", bufs=2) as ctpool,
            tc.tile_pool(name="cnp", bufs=2) as cnpool,
            tc.tile_pool(name="qfp", bufs=2) as qfpool,
            tc.tile_pool(name="qwp", bufs=2) as qwpool,
            tc.tile_pool(name="e2p", bufs=20) as e2pool,
            tc.tile_pool(name="e2tp", bufs=6) as e2tpool,
            tc.tile_pool(name="tfp", bufs=6) as tfpool,
            tc.tile_pool(name="smallp", bufs=24) as smallpool,
            tc.tile_pool(name="stagep", bufs=4) as stagepool,
            tc.tile_pool(name="psbig", bufs=4, space="PSUM") as psb,
            tc.tile_pool(name="pstr", bufs=2, space="PSUM") as psbt,
            tc.tile_pool(name="pssmall", bufs=2, space="PSUM") as pss,
        ):
            ident = constp.tile([128, 128], bf16, name="ident")
            nc.sync.dma_start(ident[:], id_d[:])
            zb = constp.tile([128, BPC, NT], bf16, name="zb")
            nc.sync.dma_start(zb[:], zb_d[:])
            c1m = constp.tile([128, BPC, NT], f32, name="c1m")
            nc.sync.dma_start(c1m[:], c1m_d[:])
            f1f = constp.tile([128, BPC, MT], f32, name="f1f")
            nc.sync.dma_start(f1f[:], f1f_d[:])
            f1b = constp.tile([128, BPC, MT], bf16, name="f1b")
            nc.sync.dma_start(f1b[:], f1b_d[:])

            for b in [b for _ in range(n_reps) for b in range(BPC)]:
                ct = ctpool.tile([128, DT, N], bf16, name="CT", tag="CT")
                nc.sync.dma_start(
                    ct[:], CT_d[b].rearrange("(j p) n -> p j n", p=128)
                )
                cn = cnpool.tile([128, NT, D], bf16, name="Cn", tag="Cn")
                nc.sync.dma_start(
                    cn[:], Cn_d[b].rearrange("(s p) d -> p s d", p=128)
                )
                qf = qfpool.tile([128, MT, D], bf16, name="Qf", tag="Qf")
                nc.sync.dma_start(
                    qf[:], Qf_d[b].rearrange("(s p) d -> p s d", p=128)
                )
                qwt = qwpool.tile([128, DT, M], bf16, name="QwT", tag="QwT")
                nc.sync.dma_start(
                    qwt[:], QwT_d[b].rearrange("(j p) m -> p j m", p=128)
                )

                # ---- E2[t] = exp(dot3 + c1m[n]) [16 x [128n, 512m] bf16],
                # with E2T transposes of group tq-1 interleaved behind the
                # dot3 matmuls of group tq to keep the PE dependency-free.
                e2_tiles = [
                    e2pool.tile([128, M], bf16, name=f"E2_{t}", tag="E2")
                    for t in range(NT)
                ]
                e2t_tiles = [
                    e2tpool.tile([128, N], bf16, name=f"E2T_{u}", tag="E2T")
                    for u in range(MT)
                ]

                def tr_group(tq):
                    # transpose the 4 n-tiles of group tq into all 4 E2T tiles
                    if TR_VIA_DMA:
                        for s in range(4):
                            t = tq * 4 + s
                            for u in range(MT):
                                nc.sync.dma_start_transpose(
                                    out=e2t_tiles[u][:, t * 128 : (t + 1) * 128],
                                    in_=e2_tiles[t][:, u * 128 : (u + 1) * 128],
                                )
                        return
                    for u in range(MT):
                        pst = psbt.tile([128, 512], bf16, name="ps_tr", tag="pstr")
                        for s in range(4):
                            t = tq * 4 + s
                            nc.tensor.transpose(
                                pst[:, s * 128 : (s + 1) * 128],
                                e2_tiles[t][:, u * 128 : (u + 1) * 128],
                                ident[:],
                            )
                        nc.vector.tensor_copy(
                            e2t_tiles[u][:, tq * 512 : (tq + 1) * 512], pst[:]
                        )

                for tq in range(NQ):
                    for s in range(4):
                        t = tq * 4 + s
                        ps = psb.tile([128, M], f32, name="ps_e2", tag="psb")
                        for j in range(DT):
                            mm(
                                ps[:],
                                ct[:, j, t * 128 : (t + 1) * 128],
                                qwt[:, j, :],
                                start=(j == 0),
                                stop=(j == DT - 1),
                            )
                        nc.scalar.activation(
                            e2_tiles[t][:], ps[:], AF.Exp, bias=c1m[:, b, t : t + 1]
                        )
                    if tq:
                        tr_group(tq - 1)

                # ---- T stage: c2' = E2.T @ 1, T' = E2.T @ C;
                # Tf = T' * f1/c2' (the f2 in E2 cancels against r1f2 below)
                tf_tiles = []
                for u in range(MT):
                    if u == 0:
                        tr_group(NQ - 1)  # last transpose group, after its exps
                    pst = psb.tile([128, D], f32, name="ps_T", tag="psb")
                    psc = pss.tile([128, 1], f32, name="ps_c2", tag="pss")
                    for t in range(NT):
                        lhsT = e2_tiles[t][:, u * 128 : (u + 1) * 128]
                        mm(
                            pst[:], lhsT, cn[:, t, :],
                            start=(t == 0), stop=(t == NT - 1),
                        )
                        mm(
                            psc[:], lhsT, zb[:, b, t : t + 1],
                            start=(t == 0), stop=(t == NT - 1),
                        )
                    rc = smallpool.tile([128, 1], f32, name="rc2", tag="small")
                    nc.vector.reciprocal(rc[:], psc[:])
                    sc = smallpool.tile([128, 1], f32, name="scT", tag="small")
                    nc.vector.tensor_scalar_mul(sc[:], f1f[:, b, u : u + 1], rc[:])
                    tfu = tfpool.tile([128, D], bf16, name="Tf", tag="Tf")
                    nc.scalar.activation(tfu[:], pst[:], AF.Copy, scale=sc[:])
                    tf_tiles.append(tfu)

                # ---- A/B: A = (E2T.T @ Qf)/r1f2, B = (E2T.T @ Tf)/r1f2
                for g in range(NT // 2):
                    ast = stagepool.tile([128, 2, D], bf16, name="Ast", tag="Ast")
                    bst = stagepool.tile([128, 2, D], bf16, name="Bst", tag="Bst")
                    for s2 in range(2):
                        t = g * 2 + s2
                        psa = psb.tile([128, D], f32, name="ps_A", tag="psb")
                        psbb = psb.tile([128, D], f32, name="ps_B", tag="psb")
                        psr = pss.tile([128, 1], f32, name="ps_r1", tag="pss")
                        for u in range(MT):
                            lhsT = e2t_tiles[u][:, t * 128 : (t + 1) * 128]
                            mm(
                                psa[:], lhsT, qf[:, u, :],
                                start=(u == 0), stop=(u == MT - 1),
                            )
                            mm(
                                psbb[:], lhsT, tf_tiles[u][:],
                                start=(u == 0), stop=(u == MT - 1),
                            )
                            mm(
                                psr[:], lhsT, f1b[:, b, u : u + 1],
                                start=(u == 0), stop=(u == MT - 1),
                            )
                        r1 = smallpool.tile([128, 1], f32, name="r1", tag="small")
                        nc.vector.reciprocal(r1[:], psr[:])
                        nc.scalar.activation(
                            ast[:, s2, :], psa[:], AF.Copy, scale=r1[:]
                        )
                        nc.vector.tensor_scalar_mul(bst[:, s2, :], psbb[:], r1[:])
                    nc.sync.dma_start(
                        A_d[b, g * 256 : (g + 1) * 256, :].rearrange(
                            "(s p) d -> p s d", p=128
                        ),
                        ast[:],
                    )
                    nc.sync.dma_start(
                        Bo_d[b, g * 256 : (g + 1) * 256, :].rearrange(
                            "(s p) d -> p s d", p=128
                        ),
                        bst[:],
                    )

    return nc


_NC = None


def _get_nc():
    global _NC
    if _NC is None:
        _NC = build_nc()
        _NC.finalize()
    return _NC


def _make_in_maps(C, Q, Cmask, Qmask, w):
    import ml_dtypes

    bf = ml_dtypes.bfloat16
    C = np.asarray(C, dtype=np.float32)
    Q = np.asarray(Q, dtype=np.float32)
    w = np.asarray(w, dtype=np.float32)
    w1, w2, w3 = w[:D], w[D : 2 * D], w[2 * D :]

    c1 = C @ w1  # [B, N]
    q2 = Q @ w2  # [B, M]
    c1m_full = c1  # S1 path is NOT masked by Cmask; Cmask enters via z below
    z_full = 1.0 - np.asarray(Cmask, dtype=np.float32)  # [B, N]; 0 = masked
    f1_full = np.exp(q2 + np.float32(NEG) * np.asarray(Qmask, dtype=np.float32))

    Cb = C.astype(bf)
    CTb = np.ascontiguousarray(Cb.transpose(0, 2, 1))
    Czb = (z_full[:, :, None] * C).astype(bf)  # masked rows zeroed, for T path
    Qfb = (f1_full[:, :, None] * Q).astype(bf)
    QwTb = np.ascontiguousarray((Q * w3[None, None, :]).astype(bf).transpose(0, 2, 1))
    ident = np.eye(128, dtype=bf)

    in_maps = []
    for c in range(NCORES):
        bs = slice(c * BPC, (c + 1) * BPC)
        c1m = np.ascontiguousarray(
            c1m_full[bs].reshape(BPC, NT, 128).transpose(2, 0, 1)
        )
        zl = z_full[bs].reshape(BPC, NT, 128).transpose(2, 0, 1)
        f1l = f1_full[bs].reshape(BPC, MT, 128).transpose(2, 0, 1)
        in_maps.append(
            {
                "CT": CTb[bs],
                "Cn": np.ascontiguousarray(Czb[bs]),
                "Qf": np.ascontiguousarray(Qfb[bs]),
                "QwT": QwTb[bs],
                "c1m": c1m,
                "f1f": np.ascontiguousarray(f1l.astype(np.float32)),
                "f1b": np.ascontiguousarray(f1l.astype(bf)),
                "ident": ident,
                "zb": np.ascontiguousarray(zl.astype(bf)),
            }
        )
    return in_maps


def run_spmd(C, Q, Cmask, Qmask, w, trace=False):
    """Returns ((A, Bout), BassKernelResults)."""
    from concourse.bass_utils import run_bass_kernel_spmd

    nc = _get_nc()
    in_maps = _make_in_maps(C, Q, Cmask, Qmask, w)
    res = run_bass_kernel_spmd(nc, in_maps, list(range(NCORES)), trace=trace)
    A = np.concatenate(
        [np.asarray(r["A"]).astype(np.float32) for r in res.results], axis=0
    )
    Bout = np.concatenate(
        [np.asarray(r["Bout"]).astype(np.float32) for r in res.results], axis=0
    )
    return (A, Bout), res


def kernel(C, Q, Cmask, Qmask, w):
    # NTFF tracing is unavailable under this container's axon relay; always
    # run the plain execute path.
    (A, Bout), _ = run_spmd(C, Q, Cmask, Qmask, w, trace=False)
    return (A, Bout)
